# revision 21
# baseline (speedup 1.0000x reference)
"""CGConv (gnn_message_passing) Trainium2 kernel — 8-core SPMD, v2.

Strategy vs v1 baseline:
- Edges sorted by (owner core of src, dst-half, src-group); each core owns a
  contiguous 6272-node range so segment sums are core-local.
- Only x[dst] is gathered (SWDGE dma_gather from DRAM half-tables). The
  x[src]@W1 term is produced by one-hot matmuls against a precomputed
  U = x_local @ W1 table (src-sorted blocks hit few node groups), which
  removes half of the descriptor-generation load on GpSimd (the bottleneck).
- Slots are packed densely with uniform-across-cores sizes; 128-edge blocks
  may span two slots, handled by +128-per-segment value encoding in the
  one-hot builders.
- Phase 2 applies BN1+sigmoid/softplus feature-major (per-partition fused
  scale/bias on the Act engine), then one x-bar DMA-transpose per megabatch
  yields edge-major messages for the PE scatter (no per-block transposes).
- BatchNorm stats are all-reduced (tiny) across cores.
"""

import sys

sys.path.insert(0, "/opt/trn_rl_repo")

import numpy as np
import ml_dtypes

from concourse import bass, bacc, tile, mybir
from concourse import bass_utils

BF16 = ml_dtypes.bfloat16
FP16 = np.float16

# Problem constants (hardcoded per harness contract)
N, E, ATOM, NBR = 50000, 800000, 128, 64
DOUT = 2 * ATOM  # 256
BN_EPS = 1e-5

NCORES = 8
GPC = 49                      # groups of 128 nodes per core
NPC = GPC * 128               # 6272 nodes per core
NPAD = NCORES * NPC           # 50176
HALF = NPAD // 2              # 25088 node-table half split (int16 idx limit)
GB = 2048                     # phase-1 batch (edges)
MB = 4096                     # phase-2 mega-batch (edges)

_DT = mybir.dt


def _ceil(a, b):
    return -(-a // b)


def _wrap_idx(idx):
    """[n] -> [128, n//16] int16, wrapped in 16 partitions, replicated x8."""
    n = idx.shape[0]
    w = idx.reshape(n // 16, 16).T.astype(np.int16)  # [16, n//16]
    return np.tile(w, (8, 1))


def _prep(x, edge_index, edge_attr):
    """Host-side sharding: sort edges into a uniform per-core dense layout."""
    src = np.asarray(edge_index[0], dtype=np.int64)
    dst = np.asarray(edge_index[1], dtype=np.int64)
    ea = np.asarray(edge_attr, dtype=np.float32)

    g = src >> 7                      # node group 0..391
    core = g // GPC
    gi = g % GPC
    sub = (dst >= HALF).astype(np.int64)
    key = (core * 2 + sub) * GPC + gi
    order = np.argsort(key, kind="stable")
    src_s, dst_s = src[order], dst[order]
    ea_s = ea[order].astype(BF16)

    counts = np.bincount(key, minlength=NCORES * 2 * GPC).reshape(NCORES, 2, GPC)
    S = counts.max(axis=0)                      # [2, GPC] uniform slot sizes
    L0 = int(S[0].sum())
    L1 = int(S[1].sum())
    L0p = _ceil(max(L0, 1), GB) * GB
    L1p = _ceil(max(L1, 1), GB) * GB
    e_pad = L0p + L1p
    nblk = e_pad // 128

    # uniform slot start positions
    slot_start = np.zeros((2, GPC), dtype=np.int64)
    pos = 0
    for q in range(GPC):
        slot_start[0, q] = pos
        pos += int(S[0, q])
    pos = L0p
    for q in range(GPC):
        slot_start[1, q] = pos
        pos += int(S[1, q])

    # per-position segment index within its block (uniform across cores):
    # block gb spans positions [gb*128, gb*128+128); slots intersecting it
    # get consecutive segment indices 0,1,...
    # Also build per-block segment program: list over blocks of
    # (q, k, first_of_slot, last_of_slot, sub)
    seg_of_pos = np.full(e_pad, -1, dtype=np.int64)
    blocks = [[] for _ in range(nblk)]
    for s in range(2):
        for q in range(GPC):
            a = int(slot_start[s, q])
            b = a + int(S[s, q])
            if b == a:
                continue
            gb0, gb1 = a // 128, (b - 1) // 128
            for gb in range(gb0, gb1 + 1):
                lo = max(a, gb * 128)
                hi = min(b, (gb + 1) * 128)
                k = len(blocks[gb])
                seg_of_pos[lo:hi] = k
                blocks[gb].append(dict(
                    q=q, k=k, sub=s,
                    first=(gb == gb0), last=(gb == gb1),
                ))
    max_k = max((len(b) for b in blocks), default=1)
    assert max_k * 128 <= 2040, f"segment offset overflow: {max_k}"

    # per-core data
    slot_off = np.zeros(NCORES * 2 * GPC + 1, dtype=np.int64)
    np.cumsum(counts.reshape(-1), out=slot_off[1:])
    ZHALF = HALF  # zero-token row index in each half table

    per_core = []
    for c in range(NCORES):
        xj_idx = np.full(e_pad, ZHALF, dtype=np.int16)
        slb = np.full(e_pad, -1.0, dtype=np.float32)
        eat = np.zeros((e_pad, NBR), dtype=BF16)
        for s in range(2):
            for q in range(GPC):
                kk = (c * 2 + s) * GPC + q
                a, b_ = slot_off[kk], slot_off[kk + 1]
                n = b_ - a
                if n == 0:
                    continue
                o = int(slot_start[s, q])
                xj_idx[o:o + n] = (dst_s[a:b_] - s * HALF).astype(np.int16)
                slb[o:o + n] = (src_s[a:b_] & 127).astype(np.float32) \
                    + 128.0 * seg_of_pos[o:o + n]
                eat[o:o + n] = ea_s[a:b_]
        per_core.append(
            dict(
                xj_idx=_wrap_idx(xj_idx),
                slbT=np.ascontiguousarray(slb[None, :]),      # [1, e_pad]
                slbP=np.ascontiguousarray(
                    slb.reshape(nblk, 128).T.astype(FP16)),   # [128, nblk]
                eaT=np.ascontiguousarray(eat.T),
            )
        )

    x_bf = np.zeros((NPAD, ATOM), dtype=BF16)
    x_bf[:N] = np.asarray(x, dtype=np.float32).astype(BF16)
    xa = np.concatenate([x_bf[:HALF], np.zeros((1, ATOM), BF16)])   # [HALF+1,128]
    xb = np.concatenate([x_bf[HALF:], np.zeros((1, ATOM), BF16)])
    for c in range(NCORES):
        per_core[c]["xa"] = xa
        per_core[c]["xb"] = xb
        per_core[c]["xlocT"] = np.ascontiguousarray(
            x_bf[c * NPC:(c + 1) * NPC].T)                          # [128, NPC]

    struct = dict(e_pad=e_pad, L0p=L0p, nblk=nblk, blocks=blocks, max_k=max_k)
    return per_core, struct


def _build(struct):
    """Build the 8-core SPMD bass program. Returns compiled Bacc."""
    e_pad = struct["e_pad"]
    L0p = struct["L0p"]
    nblk = struct["nblk"]
    blocks = struct["blocks"]
    max_k = max(2, struct["max_k"])

    nc = bacc.Bacc("TRN2", target_bir_lowering=False, debug=False,
                   num_devices=NCORES)
    f32, bf16, fp16, i16 = _DT.float32, _DT.bfloat16, _DT.float16, _DT.int16

    def din(name, shape, dt):
        return nc.dram_tensor(name, shape, dt, kind="ExternalInput").ap()

    xj_idx_d = din("xj_idx", [128, e_pad // 16], i16)
    slbP_d = din("slbP", [128, nblk], fp16)
    iden_d = din("iden", [128, 128], fp16)
    eaT_d = din("eaT", [NBR, e_pad], bf16)
    xa_d = din("xa", [HALF + 1, ATOM], bf16)
    xb_d = din("xb", [HALF + 1, ATOM], bf16)
    xlocT_d = din("xlocT", [128, NPC], bf16)
    wt_d = din("wt", [320, DOUT], bf16)
    g1b1_d = din("g1b1", [128, 4], f32)
    g2b2_d = din("g2b2", [1, 2 * ATOM], f32)
    iotac_d = din("iotac", [128, max_k], f32)       # col n -> n + 128k
    iotar_d = din("iotar", [128, max_k * 128], fp16)  # row e -> n + 128k
    ones_c_d = din("ones_col", [128, 1], f32)
    ones_r_d = din("ones_row", [1, 128], f32)
    y_d = nc.dram_tensor("y", [NPC, ATOM], f32, kind="ExternalOutput").ap()

    AF = mybir.ActivationFunctionType
    ALU = mybir.AluOpType
    nbatch = e_pad // GB

    with tile.TileContext(nc) as tc:
        with (
            tc.tile_pool(name="const", bufs=1) as cp,
            tc.tile_pool(name="dram", bufs=1, space="DRAM") as dram,
        ):
            # persistent SBUF state
            w_sb = cp.tile([128, 2 * DOUT], bf16, tag="w")    # W1 | W2
            w2_sb = cp.tile([64, DOUT], bf16, tag="w2")       # W3 (ea)
            nc.scalar.dma_start(w_sb[:, 0:DOUT], wt_d[0:128, :])
            nc.scalar.dma_start(w_sb[:, DOUT:2 * DOUT], wt_d[128:256, :])
            nc.scalar.dma_start(w2_sb[:], wt_d[256:320, :])
            iden_sb = cp.tile([128, 128], fp16, tag="iden")
            nc.scalar.dma_start(iden_sb[:], iden_d[:])
            slbP_sb = cp.tile([128, nblk], fp16, tag="slbP")
            nc.scalar.dma_start(slbP_sb[:], slbP_d[:])
            iotac_sb = cp.tile([128, max_k], f32, tag="iotac")
            nc.scalar.dma_start(iotac_sb[:], iotac_d[:])
            iotar_sb = cp.tile([128, max_k * 128], fp16, tag="iotar")
            nc.scalar.dma_start(iotar_sb[:], iotar_d[:])
            g1b1_sb = cp.tile([128, 4], f32, tag="g1b1")
            nc.scalar.dma_start(g1b1_sb[:], g1b1_d[:])
            g2b2_sb = cp.tile([1, 2 * ATOM], f32, tag="g2b2")
            nc.scalar.dma_start(g2b2_sb[:], g2b2_d[:])
            ones_c = cp.tile([128, 1], f32, tag="onesc")
            nc.scalar.dma_start(ones_c[:], ones_c_d[:])
            ones_r = cp.tile([1, 128], f32, tag="onesr")
            nc.scalar.dma_start(ones_r[:], ones_r_d[:])

            summed = cp.tile([128, GPC * 128], f32, tag="summed")
            nc.vector.memset(summed[:], 0.0)

            U_sb = cp.tile([128, GPC * DOUT], bf16, tag="U")
            mv_f = cp.tile([128, 2 * nbatch], f32, tag="mvf")
            mv_c = cp.tile([128, 2 * nbatch], f32, tag="mvc")
            s1 = cp.tile([128, 2], f32, tag="s1")
            t1 = cp.tile([128, 2], f32, tag="t1")
            ns1 = cp.tile([128, 1], f32, tag="ns1")
            nt1 = cp.tile([128, 1], f32, tag="nt1")

            spill_f = dram.tile([128, e_pad], bf16)
            spill_c = dram.tile([128, e_pad], bf16)
            sigsp = dram.tile([128, e_pad], bf16)

            # ---------------- PHASE 0: U = x_loc @ W1 ----------------
            with (
                tc.tile_pool(name="u0", bufs=2) as up,
                tc.tile_pool(name="psu", bufs=2, space="PSUM") as pu,
            ):
                xlocT_sb = up.tile([128, NPC], bf16, tag="xlT")
                nc.scalar.dma_start(xlocT_sb[:], xlocT_d[:])
                for q in range(GPC):
                    psU = pu.tile([128, DOUT], f32, tag="psU")
                    nc.tensor.matmul(psU[:], xlocT_sb[:, q * 128:(q + 1) * 128],
                                     w_sb[:, 0:DOUT], start=True, stop=True)
                    nc.scalar.copy(U_sb[:, q * DOUT:(q + 1) * DOUT], psU[:])

            # ---------------- PHASE 1 ----------------
            with (
                tc.tile_pool(name="g1", bufs=2) as gp,
                tc.tile_pool(name="oh", bufs=2) as ohp,
                tc.tile_pool(name="st1", bufs=2) as sp,
                tc.tile_pool(name="ps1", bufs=1, space="PSUM") as pp,
            ):
                for bi in range(nbatch):
                    b0 = bi * GB
                    xj_tab = xa_d if b0 < L0p else xb_d
                    ixj = gp.tile([128, GB // 16], i16, tag="ixj")
                    nc.sync.dma_start(ixj[:], xj_idx_d[:, b0 // 16:(b0 + GB) // 16])
                    xjT = gp.tile([128, 1, GB], bf16, tag="xjT")
                    # ucode limit: dma_gather dies above 768 idxs; use 512
                    for p in range(GB // 512):
                        nc.gpsimd.dma_gather(
                            out_ap=xjT[:, :, p * 512:(p + 1) * 512],
                            in_ap=xj_tab[:], idxs_ap=ixj[:, p * 32:(p + 1) * 32],
                            num_idxs=512, num_idxs_reg=512, elem_size=128,
                            transpose=True,
                        )
                    eat = gp.tile([NBR, GB], bf16, tag="eat")
                    nc.sync.dma_start(eat[:], eaT_d[:, b0:b0 + GB])

                    ps = [pp.tile([128, GB], f32, name=f"ps{h}", tag=f"ps{h}")
                          for h in range(2)]
                    # PE broadcast-transposes put slb values (per edge, down
                    # partitions) into psum half 0; one wide strided is_equal
                    # per segment offset builds all 16 blocks' one-hot
                    # transposes at once.
                    for c in range(GB // 128):
                        gb = b0 // 128 + c
                        reg = ps[0][:, c * 128:(c + 1) * 128].bitcast(fp16)[:, 0:128]
                        nc.tensor.transpose(
                            reg,
                            slbP_sb[:, gb:gb + 1].to_broadcast([128, 128]),
                            iden_sb[:],
                        )
                    segs_of = [blocks[b0 // 128 + c] or
                               [dict(q=0, k=0, sub=0, first=False, last=False)]
                               for c in range(GB // 128)]
                    kmax = max(len(sgl) for sgl in segs_of)
                    psv = ps[0].bitcast(fp16).rearrange(
                        "p (m l) -> p m l", l=256)[:, :, 0:128]
                    ohT = []
                    for k in range(kmax):
                        oh = ohp.tile([128, GB // 128, 128], bf16,
                                      name=f"ohT{k}", tag=f"ohT{k}")
                        nc.vector.tensor_scalar(
                            oh[:], psv, iotac_sb[:, k:k + 1], None, ALU.is_equal)
                        ohT.append(oh)

                    # gated matmuls, chunk order per piece: ea (start), xj,
                    # k>=1 xi singles, then k=0 xi groups (merged over
                    # consecutive blocks sharing a slot) carrying stop=True.
                    for h in range(2):
                        for p in range(4):
                            reg = ps[h][:, p * 512:(p + 1) * 512]
                            nc.tensor.matmul(
                                reg, w2_sb[:, h * 128:(h + 1) * 128],
                                eat[:, p * 512:(p + 1) * 512],
                                start=True, stop=False)
                            nc.tensor.matmul(
                                reg,
                                w_sb[:, DOUT + h * 128:DOUT + (h + 1) * 128],
                                xjT[:, 0, p * 512:(p + 1) * 512],
                                start=False, stop=False)
                            for c in range(p * 4, p * 4 + 4):
                                for sg in segs_of[c][1:]:
                                    nc.tensor.matmul(
                                        ps[h][:, c * 128:(c + 1) * 128],
                                        U_sb[:, sg["q"] * DOUT + h * 128:
                                             sg["q"] * DOUT + (h + 1) * 128],
                                        ohT[sg["k"]][:, c, :],
                                        start=False, stop=False)
                            ca = p * 4
                            while ca < p * 4 + 4:
                                q0 = segs_of[ca][0]["q"]
                                cb = ca + 1
                                while cb < p * 4 + 4 and segs_of[cb][0]["q"] == q0:
                                    cb += 1
                                nc.tensor.matmul(
                                    ps[h][:, ca * 128:cb * 128],
                                    U_sb[:, q0 * DOUT + h * 128:
                                         q0 * DOUT + (h + 1) * 128],
                                    ohT[0][:, ca:cb, :],
                                    start=False, stop=True)
                                ca = cb

                    bst = sp.tile([128, 2, 24], f32, tag="bst")
                    stage_f = sp.tile([128, GB], bf16, tag="stf")
                    stage_c = sp.tile([128, GB], bf16, tag="stc")
                    for h, (stg, spl, mv) in enumerate((
                            (stage_f, spill_f, mv_f), (stage_c, spill_c, mv_c))):
                        nc.scalar.copy(stg[:], ps[h][:])
                        for p in range(4):
                            nc.vector.bn_stats(bst[:, h, p * 6:(p + 1) * 6],
                                               stg[:, p * 512:(p + 1) * 512])
                        nc.vector.bn_aggr(mv[:, 2 * bi:2 * bi + 2], bst[:, h, :])
                        nc.sync.dma_start(spl[:, b0:b0 + GB], stg[:])

            # ---------------- BN1 stats all-reduce ----------------
            # per-batch (mean, var) with equal counts (GB each, pads are zero):
            # sum = GB * sum(means); sumsq = GB * sum(var + mean^2)
            st_loc = cp.tile([128, 4], f32, tag="stloc")
            ex2b = cp.tile([128, 2 * nbatch], f32, tag="ex2b")
            for h, mv in enumerate((mv_f, mv_c)):
                means = mv.rearrange("p (n k) -> p k n", k=2)[:, 0, :]
                varls = mv.rearrange("p (n k) -> p k n", k=2)[:, 1, :]
                m2 = ex2b[:, h * nbatch:(h + 1) * nbatch]
                nc.vector.tensor_tensor(m2, means, means, ALU.mult)
                nc.vector.tensor_tensor(m2, m2, varls, ALU.add)
                nc.vector.tensor_reduce(st_loc[:, h:h + 1], means,
                                        mybir.AxisListType.X, ALU.add)
                nc.vector.tensor_reduce(st_loc[:, 2 + h:3 + h], m2,
                                        mybir.AxisListType.X, ALU.add)
            nc.vector.tensor_scalar_mul(st_loc[:], st_loc[:], float(GB))
            st_in = dram.tile([128, 4], f32)
            st_out = dram.tile([128, 4], f32)
            nc.gpsimd.dma_start(st_in[:], st_loc[:])
            nc.gpsimd.collective_compute(
                "AllReduce", ALU.add, replica_groups=[list(range(NCORES))],
                ins=[st_in.opt()], outs=[st_out.opt()],
            )
            st_g = cp.tile([128, 4], f32, tag="stg")
            nc.gpsimd.dma_start(st_g[:], st_out[:])
            # mean/var -> affine s1, t1  (b cancels in BN; never added)
            mv = cp.tile([128, 6], f32, tag="mv")
            nc.vector.tensor_scalar_mul(mv[:, 0:2], st_g[:, 0:2], 1.0 / E)
            nc.vector.tensor_scalar_mul(mv[:, 2:4], st_g[:, 2:4], 1.0 / E)
            nc.vector.tensor_tensor(mv[:, 4:6], mv[:, 0:2], mv[:, 0:2], ALU.mult)
            nc.vector.tensor_tensor(mv[:, 2:4], mv[:, 2:4], mv[:, 4:6], ALU.subtract)
            nc.vector.tensor_scalar_add(mv[:, 2:4], mv[:, 2:4], float(BN_EPS))
            std = cp.tile([128, 2], f32, tag="std")
            nc.scalar.activation(std[:], mv[:, 2:4], AF.Sqrt, bias=0.0)
            rstd = cp.tile([128, 2], f32, tag="rstd")
            nc.vector.reciprocal(rstd[:], std[:])
            nc.vector.tensor_tensor(s1[:], g1b1_sb[:, 0:2], rstd[:], ALU.mult)
            nc.vector.tensor_tensor(mv[:, 4:6], mv[:, 0:2], s1[:], ALU.mult)
            nc.vector.tensor_tensor(t1[:], g1b1_sb[:, 2:4], mv[:, 4:6], ALU.subtract)
            nc.vector.tensor_scalar_mul(ns1[:], s1[:, 0:1], -1.0)
            nc.vector.tensor_scalar_mul(nt1[:], t1[:, 0:1], -1.0)

            # ---------------- PHASE 2 ----------------
            with (
                tc.tile_pool(name="ga", bufs=3) as rpa,
                tc.tile_pool(name="sa", bufs=3) as spa,
            ):
                # -------- phase 2 pass A: sigmoid only (one act table) -----
                for m0 in range(0, e_pad, MB):
                    msz = min(MB, e_pad - m0)
                    gf = rpa.tile([128, MB], bf16, tag="gf")
                    nc.sync.dma_start(gf[:, :msz], spill_f[:, m0:m0 + msz])
                    sig = spa.tile([128, MB], bf16, tag="sig")
                    nc.scalar.activation(sig[:, :msz], gf[:, :msz], AF.Sigmoid,
                                         bias=t1[:, 0:1], scale=s1[:, 0:1])
                    nc.sync.dma_start(sigsp[:, m0:m0 + msz], sig[:, :msz])

            # -------- phase 2 pass B: exp/ln, product, scatter ------------
            with (
                tc.tile_pool(name="g2", bufs=3) as rp,
                tc.tile_pool(name="m2", bufs=2) as mp,
                tc.tile_pool(name="me", bufs=2) as ep,
                tc.tile_pool(name="oh2", bufs=2) as oh2,
                tc.tile_pool(name="psg", bufs=2, space="PSUM") as pg_pool,
                tc.tile_pool(name="sq2", bufs=2) as sq2p,
                tc.tile_pool(name="ps2", bufs=1, space="PSUM") as pq,
            ):
                ps_g = None
                ps_sum = pq.tile([1, 128], f32, tag="pssum")
                ps_ssq = pq.tile([1, 128], f32, tag="psssq")
                last_sub = {}
                for gb in range(nblk):
                    for sg in blocks[gb]:
                        if sg["last"]:
                            last_sub[sg["q"]] = sg["sub"]
                nq_emit = len(last_sub)
                qcount = [0]
                for m0 in range(0, e_pad, MB):
                    msz = min(MB, e_pad - m0)
                    gc = rp.tile([128, MB], bf16, tag="gc")
                    nc.sync.dma_start(gc[:, :msz], spill_c[:, m0:m0 + msz])
                    sigl = rp.tile([128, MB], bf16, tag="sigl")
                    nc.sync.dma_start(sigl[:, :msz], sigsp[:, m0:m0 + msz])
                    # one-hots first: no barrier dependency, DVE runs ahead
                    chunks = []
                    for c0 in range(0, msz // 128, 16):
                        gb0 = m0 // 128 + c0
                        nch = min(16, msz // 128 - c0)
                        kmax2 = max((len(blocks[gb0 + i]) for i in range(nch)),
                                    default=0)
                        ohs = []
                        for k in range(kmax2):
                            oneh = oh2.tile([128, 16, 128], bf16,
                                            name=f"oh{(c0 // 16) % 2}_{k}",
                                            tag=f"oh{(c0 // 16) % 2}_{k}")
                            nc.vector.tensor_tensor(
                                oneh[:, :nch, :],
                                iotar_sb[:, k * 128:(k + 1) * 128]
                                .rearrange("p (o l) -> p o l", o=1)
                                .to_broadcast([128, nch, 128]),
                                slbP_sb[:, gb0:gb0 + nch]
                                .to_broadcast([128, nch, 128]),
                                ALU.is_equal)
                            ohs.append(oneh)
                        chunks.append((c0, nch, ohs))
                    ec = mp.tile([128, MB], bf16, tag="ec")
                    nc.scalar.activation(ec[:, :msz], gc[:, :msz], AF.Exp,
                                         bias=t1[:, 1:2], scale=s1[:, 1:2])
                    # softplus = Ln(Exp(y) + 1), written over gc
                    nc.scalar.activation(gc[:, :msz], ec[:, :msz], AF.Ln, bias=1.0)
                    msgT = ec  # reuse
                    nc.vector.tensor_tensor(msgT[:, :msz], sigl[:, :msz],
                                            gc[:, :msz], ALU.mult)
                    msgE = ep.tile([128, MB // 128, 128], bf16, tag="msgE")
                    nc.sync.dma_start_transpose(msgE[:, :msz // 128, :],
                                                msgT[:, :msz])
                    for c0, nch, ohs in chunks:
                        for ci in range(nch):
                            gb = m0 // 128 + c0 + ci
                            for sg in blocks[gb]:
                                if sg["first"]:
                                    ps_g = pg_pool.tile([128, 128], f32,
                                                        tag="psg")
                                nc.tensor.matmul(
                                    ps_g[:], ohs[sg["k"]][:, ci, :],
                                    msgE[:, c0 + ci, :],
                                    start=sg["first"], stop=sg["last"])
                                if sg["last"]:
                                    q = sg["q"]
                                    nc.vector.tensor_tensor(
                                        summed[:, q * 128:(q + 1) * 128],
                                        summed[:, q * 128:(q + 1) * 128],
                                        ps_g[:], ALU.add)
                                    if last_sub[q] == sg["sub"]:
                                        sq = sq2p.tile([128, 128], f32,
                                                       tag="sq")
                                        nc.vector.tensor_tensor(
                                            sq[:],
                                            summed[:, q * 128:(q + 1) * 128],
                                            summed[:, q * 128:(q + 1) * 128],
                                            ALU.mult)
                                        i = qcount[0]; qcount[0] += 1
                                        nc.tensor.matmul(
                                            ps_sum[:], ones_c[:],
                                            summed[:, q * 128:(q + 1) * 128],
                                            start=(i == 0),
                                            stop=(i == nq_emit - 1))
                                        nc.tensor.matmul(
                                            ps_ssq[:], ones_c[:], sq[:],
                                            start=(i == 0),
                                            stop=(i == nq_emit - 1))

                # ---------------- BN2 finalize ----------------
                st2 = cp.tile([1, 256], f32, tag="st2")
                nc.scalar.copy(st2[:, 0:128], ps_sum[:])
                nc.scalar.copy(st2[:, 128:256], ps_ssq[:])
                st2_in = dram.tile([1, 256], f32)
                st2_out = dram.tile([1, 256], f32)
                nc.gpsimd.dma_start(st2_in[:], st2[:])
                nc.gpsimd.collective_compute(
                    "AllReduce", ALU.add, replica_groups=[list(range(NCORES))],
                    ins=[st2_in.opt()], outs=[st2_out.opt()],
                )
                st2g = cp.tile([1, 256], f32, tag="st2g")
                nc.gpsimd.dma_start(st2g[:], st2_out[:])
                mv2 = cp.tile([1, 384], f32, tag="mv2")
                nc.vector.tensor_scalar_mul(mv2[:, 0:256], st2g[:], 1.0 / N)
                nc.vector.tensor_tensor(mv2[:, 256:384], mv2[:, 0:128],
                                        mv2[:, 0:128], ALU.mult)
                nc.vector.tensor_tensor(mv2[:, 128:256], mv2[:, 128:256],
                                        mv2[:, 256:384], ALU.subtract)
                nc.vector.tensor_scalar_add(mv2[:, 128:256], mv2[:, 128:256],
                                            float(BN_EPS))
                std2 = cp.tile([1, 128], f32, tag="std2")
                nc.scalar.activation(std2[:], mv2[:, 128:256], AF.Sqrt, bias=0.0)
                rstd2 = cp.tile([1, 128], f32, tag="rstd2")
                nc.vector.reciprocal(rstd2[:], std2[:])
                strow = cp.tile([1, 256], f32, tag="strow")
                nc.vector.tensor_tensor(strow[:, 0:128], g2b2_sb[:, 0:128],
                                        rstd2[:], ALU.mult)
                nc.vector.tensor_tensor(mv2[:, 256:384], mv2[:, 0:128],
                                        strow[:, 0:128], ALU.mult)
                nc.vector.tensor_tensor(strow[:, 128:256], g2b2_sb[:, 128:256],
                                        mv2[:, 256:384], ALU.subtract)
                ps_bc = pq.tile([128, 256], f32, tag="psbc")
                nc.tensor.matmul(ps_bc[:], ones_r[:], strow[:], start=True, stop=True)
                s2t2 = cp.tile([128, 256], f32, tag="s2t2")
                nc.scalar.copy(s2t2[:], ps_bc[:])
                y3 = y_d.rearrange("(q p) f -> p q f", p=128)
                sm3 = summed.rearrange("p (q l) -> p q l", l=128)
                for q0 in range(0, GPC, 7):
                    og = sq2p.tile([128, 7, 128], f32, name="og", tag="og")
                    nc.vector.tensor_tensor(
                        og[:], sm3[:, q0:q0 + 7, :],
                        s2t2[:, 0:128].rearrange("p (o l) -> p o l", o=1)
                        .to_broadcast([128, 7, 128]), ALU.mult)
                    nc.vector.tensor_tensor(
                        og[:], og[:],
                        s2t2[:, 128:256].rearrange("p (o l) -> p o l", o=1)
                        .to_broadcast([128, 7, 128]), ALU.add)
                    nc.sync.dma_start(y3[:, q0:q0 + 7, :], og[:])
    nc.compile()
    return nc


def _make_in_maps(per_core, struct, inputs):
    max_k = max(2, struct["max_k"])
    g1 = np.asarray(inputs["gamma1"], np.float32).reshape(2, 128).T  # [128,2]
    b1 = np.asarray(inputs["beta1"], np.float32).reshape(2, 128).T
    g1b1 = np.ascontiguousarray(np.concatenate([g1, b1], axis=1))  # [128,4]
    g2b2 = np.concatenate([np.asarray(inputs["gamma2"], np.float32),
                           np.asarray(inputs["beta2"], np.float32)]).reshape(1, 256)
    iotac = (np.arange(128, dtype=np.float32)[:, None]
             + 128.0 * np.arange(max_k, dtype=np.float32)[None, :])
    iotar = np.tile(np.arange(max_k * 128, dtype=np.float32), (128, 1)).astype(FP16)
    shared = dict(
        wt=np.asarray(inputs["W"], np.float32).astype(BF16),
        g1b1=g1b1,
        g2b2=np.ascontiguousarray(g2b2),
        iden=np.eye(128, dtype=FP16),
        iotac=np.ascontiguousarray(iotac),
        iotar=np.ascontiguousarray(iotar),
        ones_col=np.ones((128, 1), np.float32),
        ones_row=np.ones((1, 128), np.float32),
    )
    return [{**pc, **shared} for pc in per_core]


def kernel(x, edge_index, edge_attr, W, b, gamma1, beta1, gamma2, beta2):
    per_core, struct = _prep(x, edge_index, edge_attr)
    in_maps = _make_in_maps(
        per_core, struct,
        dict(W=W, gamma1=gamma1, beta1=beta1, gamma2=gamma2, beta2=beta2),
    )
    nc = _build(struct)
    res = bass_utils.run_bass_kernel_spmd(nc, in_maps, core_ids=list(range(NCORES)))
    out = np.concatenate([res.results[c]["y"] for c in range(NCORES)], axis=0)
    return np.ascontiguousarray(out[:N])


if __name__ == "__main__":
    import reference

    inputs = {k: np.asarray(v) for k, v in reference.setup_inputs().items()}
    got = kernel(**inputs)
    exp = np.asarray(reference.reference(**inputs))
    err = np.abs(got - exp).max() / np.abs(exp).max()
    print("rel err:", err)


# revision 22
# speedup vs baseline: 1.0982x; 1.0982x over previous
"""CGConv (gnn_message_passing) Trainium2 kernel — 8-core SPMD, v2.

Strategy vs v1 baseline:
- Edges sorted by (owner core of src, dst-half, src-group); each core owns a
  contiguous 6272-node range so segment sums are core-local.
- Only x[dst] is gathered (SWDGE dma_gather from DRAM half-tables). The
  x[src]@W1 term is produced by one-hot matmuls against a precomputed
  U = x_local @ W1 table (src-sorted blocks hit few node groups), which
  removes half of the descriptor-generation load on GpSimd (the bottleneck).
- Slots are packed densely with uniform-across-cores sizes; 128-edge blocks
  may span two slots, handled by +128-per-segment value encoding in the
  one-hot builders.
- Phase 2 applies BN1+sigmoid/softplus feature-major (per-partition fused
  scale/bias on the Act engine), then one x-bar DMA-transpose per megabatch
  yields edge-major messages for the PE scatter (no per-block transposes).
- BatchNorm stats are all-reduced (tiny) across cores.
"""

import sys

sys.path.insert(0, "/opt/trn_rl_repo")

import numpy as np
import ml_dtypes

from concourse import bass, bacc, tile, mybir
from concourse import bass_utils

BF16 = ml_dtypes.bfloat16
FP16 = np.float16

# Problem constants (hardcoded per harness contract)
N, E, ATOM, NBR = 50000, 800000, 128, 64
DOUT = 2 * ATOM  # 256
BN_EPS = 1e-5

NCORES = 8
GPC = 49                      # groups of 128 nodes per core
NPC = GPC * 128               # 6272 nodes per core
NPAD = NCORES * NPC           # 50176
HALF = NPAD // 2              # 25088 node-table half split (int16 idx limit)
GB = 2048                     # phase-1 batch (edges)
MB = 4096                     # phase-2 mega-batch (edges)

_DT = mybir.dt


def _ceil(a, b):
    return -(-a // b)


def _wrap_idx(idx):
    """[n] -> [128, n//16] int16, wrapped in 16 partitions, replicated x8."""
    n = idx.shape[0]
    w = idx.reshape(n // 16, 16).T.astype(np.int16)  # [16, n//16]
    return np.tile(w, (8, 1))


def _prep(x, edge_index, edge_attr):
    """Host-side sharding: sort edges into a uniform per-core dense layout."""
    src = np.asarray(edge_index[0], dtype=np.int64)
    dst = np.asarray(edge_index[1], dtype=np.int64)
    ea = np.asarray(edge_attr, dtype=np.float32)

    g = src >> 7                      # node group 0..391
    core = g // GPC
    gi = g % GPC
    sub = (dst >= HALF).astype(np.int64)
    key = (core * 2 + sub) * GPC + gi
    order = np.argsort(key, kind="stable")
    src_s, dst_s = src[order], dst[order]
    ea_s = ea[order].astype(BF16)

    counts = np.bincount(key, minlength=NCORES * 2 * GPC).reshape(NCORES, 2, GPC)
    S = counts.max(axis=0)                      # [2, GPC] uniform slot sizes
    L0 = int(S[0].sum())
    L1 = int(S[1].sum())
    L0p = _ceil(max(L0, 1), GB) * GB
    L1p = _ceil(max(L1, 1), GB) * GB
    e_pad = L0p + L1p
    nblk = e_pad // 128

    # uniform slot start positions
    slot_start = np.zeros((2, GPC), dtype=np.int64)
    pos = 0
    for q in range(GPC):
        slot_start[0, q] = pos
        pos += int(S[0, q])
    pos = L0p
    for q in range(GPC):
        slot_start[1, q] = pos
        pos += int(S[1, q])

    # per-position segment index within its block (uniform across cores):
    # block gb spans positions [gb*128, gb*128+128); slots intersecting it
    # get consecutive segment indices 0,1,...
    # Also build per-block segment program: list over blocks of
    # (q, k, first_of_slot, last_of_slot, sub)
    seg_of_pos = np.full(e_pad, -1, dtype=np.int64)
    blocks = [[] for _ in range(nblk)]
    for s in range(2):
        for q in range(GPC):
            a = int(slot_start[s, q])
            b = a + int(S[s, q])
            if b == a:
                continue
            gb0, gb1 = a // 128, (b - 1) // 128
            for gb in range(gb0, gb1 + 1):
                lo = max(a, gb * 128)
                hi = min(b, (gb + 1) * 128)
                k = len(blocks[gb])
                seg_of_pos[lo:hi] = k
                blocks[gb].append(dict(
                    q=q, k=k, sub=s,
                    first=(gb == gb0), last=(gb == gb1),
                ))
    max_k = max((len(b) for b in blocks), default=1)
    assert max_k * 128 <= 2040, f"segment offset overflow: {max_k}"

    # per-core data
    slot_off = np.zeros(NCORES * 2 * GPC + 1, dtype=np.int64)
    np.cumsum(counts.reshape(-1), out=slot_off[1:])
    ZHALF = HALF  # zero-token row index in each half table

    per_core = []
    for c in range(NCORES):
        xj_idx = np.full(e_pad, ZHALF, dtype=np.int16)
        slb = np.full(e_pad, -1.0, dtype=np.float32)
        eat = np.zeros((e_pad, NBR), dtype=BF16)
        for s in range(2):
            for q in range(GPC):
                kk = (c * 2 + s) * GPC + q
                a, b_ = slot_off[kk], slot_off[kk + 1]
                n = b_ - a
                if n == 0:
                    continue
                o = int(slot_start[s, q])
                xj_idx[o:o + n] = (dst_s[a:b_] - s * HALF).astype(np.int16)
                slb[o:o + n] = (src_s[a:b_] & 127).astype(np.float32) \
                    + 128.0 * seg_of_pos[o:o + n]
                eat[o:o + n] = ea_s[a:b_]
        per_core.append(
            dict(
                xj_idx=_wrap_idx(xj_idx),
                slbT=np.ascontiguousarray(slb[None, :]),      # [1, e_pad]
                slbP=np.ascontiguousarray(
                    slb.reshape(nblk, 128).T.astype(FP16)),   # [128, nblk]
                eaT=np.ascontiguousarray(eat.T),
            )
        )

    x_bf = np.zeros((NPAD, ATOM), dtype=BF16)
    x_bf[:N] = np.asarray(x, dtype=np.float32).astype(BF16)
    xa = np.concatenate([x_bf[:HALF], np.zeros((1, ATOM), BF16)])   # [HALF+1,128]
    xb = np.concatenate([x_bf[HALF:], np.zeros((1, ATOM), BF16)])
    for c in range(NCORES):
        per_core[c]["xa"] = xa
        per_core[c]["xb"] = xb
        per_core[c]["xlocT"] = np.ascontiguousarray(
            x_bf[c * NPC:(c + 1) * NPC].T)                          # [128, NPC]

    struct = dict(e_pad=e_pad, L0p=L0p, nblk=nblk, blocks=blocks, max_k=max_k)
    return per_core, struct


def _build(struct):
    """Build the 8-core SPMD bass program. Returns compiled Bacc."""
    e_pad = struct["e_pad"]
    L0p = struct["L0p"]
    nblk = struct["nblk"]
    blocks = struct["blocks"]
    max_k = max(2, struct["max_k"])

    nc = bacc.Bacc("TRN2", target_bir_lowering=False, debug=False,
                   num_devices=NCORES)
    f32, bf16, fp16, i16 = _DT.float32, _DT.bfloat16, _DT.float16, _DT.int16

    def din(name, shape, dt):
        return nc.dram_tensor(name, shape, dt, kind="ExternalInput").ap()

    xj_idx_d = din("xj_idx", [128, e_pad // 16], i16)
    slbP_d = din("slbP", [128, nblk], fp16)
    iden_d = din("iden", [128, 128], fp16)
    eaT_d = din("eaT", [NBR, e_pad], bf16)
    xa_d = din("xa", [HALF + 1, ATOM], bf16)
    xb_d = din("xb", [HALF + 1, ATOM], bf16)
    xlocT_d = din("xlocT", [128, NPC], bf16)
    wt_d = din("wt", [320, DOUT], bf16)
    g1b1_d = din("g1b1", [128, 4], f32)
    g2b2_d = din("g2b2", [1, 2 * ATOM], f32)
    iotac_d = din("iotac", [128, max_k], f32)       # col n -> n + 128k
    iotar_d = din("iotar", [128, max_k * 128], fp16)  # row e -> n + 128k
    ones_c_d = din("ones_col", [128, 1], f32)
    ones_r_d = din("ones_row", [1, 128], f32)
    y_d = nc.dram_tensor("y", [NPC, ATOM], f32, kind="ExternalOutput").ap()

    AF = mybir.ActivationFunctionType
    ALU = mybir.AluOpType
    nbatch = e_pad // GB

    with tile.TileContext(nc) as tc:
        with (
            tc.tile_pool(name="const", bufs=1) as cp,
            tc.tile_pool(name="dram", bufs=1, space="DRAM") as dram,
        ):
            # persistent SBUF state
            w_sb = cp.tile([128, 2 * DOUT], bf16, tag="w")    # W1 | W2
            w2_sb = cp.tile([64, DOUT], bf16, tag="w2")       # W3 (ea)
            nc.scalar.dma_start(w_sb[:, 0:DOUT], wt_d[0:128, :])
            nc.scalar.dma_start(w_sb[:, DOUT:2 * DOUT], wt_d[128:256, :])
            nc.scalar.dma_start(w2_sb[:], wt_d[256:320, :])
            iden_sb = cp.tile([128, 128], fp16, tag="iden")
            nc.scalar.dma_start(iden_sb[:], iden_d[:])
            slbP_sb = cp.tile([128, nblk], fp16, tag="slbP")
            nc.scalar.dma_start(slbP_sb[:], slbP_d[:])
            iotac_sb = cp.tile([128, max_k], f32, tag="iotac")
            nc.scalar.dma_start(iotac_sb[:], iotac_d[:])
            iotar_sb = cp.tile([128, max_k * 128], fp16, tag="iotar")
            nc.scalar.dma_start(iotar_sb[:], iotar_d[:])
            g1b1_sb = cp.tile([128, 4], f32, tag="g1b1")
            nc.scalar.dma_start(g1b1_sb[:], g1b1_d[:])
            g2b2_sb = cp.tile([1, 2 * ATOM], f32, tag="g2b2")
            nc.scalar.dma_start(g2b2_sb[:], g2b2_d[:])
            ones_c = cp.tile([128, 1], f32, tag="onesc")
            nc.scalar.dma_start(ones_c[:], ones_c_d[:])
            ones_r = cp.tile([1, 128], f32, tag="onesr")
            nc.scalar.dma_start(ones_r[:], ones_r_d[:])

            summed = cp.tile([128, GPC * 128], f32, tag="summed")
            nc.vector.memset(summed[:], 0.0)

            U_sb = cp.tile([128, GPC * DOUT], bf16, tag="U")
            mv_f = cp.tile([128, 2 * nbatch], f32, tag="mvf")
            mv_c = cp.tile([128, 2 * nbatch], f32, tag="mvc")
            s1 = cp.tile([128, 2], f32, tag="s1")
            t1 = cp.tile([128, 2], f32, tag="t1")
            ns1 = cp.tile([128, 1], f32, tag="ns1")
            nt1 = cp.tile([128, 1], f32, tag="nt1")

            spill_f = dram.tile([128, e_pad], bf16)
            spill_c = dram.tile([128, e_pad], bf16)

            # ---------------- PHASE 0: U = x_loc @ W1 ----------------
            with (
                tc.tile_pool(name="u0", bufs=2) as up,
                tc.tile_pool(name="psu", bufs=2, space="PSUM") as pu,
            ):
                xlocT_sb = up.tile([128, NPC], bf16, tag="xlT")
                nc.scalar.dma_start(xlocT_sb[:], xlocT_d[:])
                for q in range(GPC):
                    psU = pu.tile([128, DOUT], f32, tag="psU")
                    nc.tensor.matmul(psU[:], xlocT_sb[:, q * 128:(q + 1) * 128],
                                     w_sb[:, 0:DOUT], start=True, stop=True)
                    nc.scalar.copy(U_sb[:, q * DOUT:(q + 1) * DOUT], psU[:])

            # ---------------- PHASE 1 ----------------
            with (
                tc.tile_pool(name="g1", bufs=2) as gp,
                tc.tile_pool(name="oh", bufs=2) as ohp,
                tc.tile_pool(name="st1", bufs=2) as sp,
                tc.tile_pool(name="ps1", bufs=1, space="PSUM") as pp,
            ):
                for bi in range(nbatch):
                    b0 = bi * GB
                    xj_tab = xa_d if b0 < L0p else xb_d
                    ixj = gp.tile([128, GB // 16], i16, tag="ixj")
                    nc.sync.dma_start(ixj[:], xj_idx_d[:, b0 // 16:(b0 + GB) // 16])
                    xjT = gp.tile([128, 1, GB], bf16, tag="xjT")
                    # ucode limit: dma_gather dies above 768 idxs; use 512
                    for p in range(GB // 512):
                        nc.gpsimd.dma_gather(
                            out_ap=xjT[:, :, p * 512:(p + 1) * 512],
                            in_ap=xj_tab[:], idxs_ap=ixj[:, p * 32:(p + 1) * 32],
                            num_idxs=512, num_idxs_reg=512, elem_size=128,
                            transpose=True,
                        )
                    eat = gp.tile([NBR, GB], bf16, tag="eat")
                    nc.sync.dma_start(eat[:], eaT_d[:, b0:b0 + GB])

                    ps = [pp.tile([128, GB], f32, name=f"ps{h}", tag=f"ps{h}")
                          for h in range(2)]
                    # PE broadcast-transposes put slb values (per edge, down
                    # partitions) into psum half 0; one wide strided is_equal
                    # per segment offset builds all 16 blocks' one-hot
                    # transposes at once.
                    for c in range(GB // 128):
                        gb = b0 // 128 + c
                        reg = ps[0][:, c * 128:(c + 1) * 128].bitcast(fp16)[:, 0:128]
                        nc.tensor.transpose(
                            reg,
                            slbP_sb[:, gb:gb + 1].to_broadcast([128, 128]),
                            iden_sb[:],
                        )
                    segs_of = [blocks[b0 // 128 + c] or
                               [dict(q=0, k=0, sub=0, first=False, last=False)]
                               for c in range(GB // 128)]
                    kmax = max(len(sgl) for sgl in segs_of)
                    psv = ps[0].bitcast(fp16).rearrange(
                        "p (m l) -> p m l", l=256)[:, :, 0:128]
                    ohT = []
                    for k in range(kmax):
                        oh = ohp.tile([128, GB // 128, 128], bf16,
                                      name=f"ohT{k}", tag=f"ohT{k}")
                        nc.vector.tensor_scalar(
                            oh[:], psv, iotac_sb[:, k:k + 1], None, ALU.is_equal)
                        ohT.append(oh)

                    # gated matmuls, chunk order per piece: ea (start), xj,
                    # k>=1 xi singles, then k=0 xi groups (merged over
                    # consecutive blocks sharing a slot) carrying stop=True.
                    for h in range(2):
                        for p in range(4):
                            reg = ps[h][:, p * 512:(p + 1) * 512]
                            nc.tensor.matmul(
                                reg, w2_sb[:, h * 128:(h + 1) * 128],
                                eat[:, p * 512:(p + 1) * 512],
                                start=True, stop=False)
                            nc.tensor.matmul(
                                reg,
                                w_sb[:, DOUT + h * 128:DOUT + (h + 1) * 128],
                                xjT[:, 0, p * 512:(p + 1) * 512],
                                start=False, stop=False)
                            for c in range(p * 4, p * 4 + 4):
                                for sg in segs_of[c][1:]:
                                    nc.tensor.matmul(
                                        ps[h][:, c * 128:(c + 1) * 128],
                                        U_sb[:, sg["q"] * DOUT + h * 128:
                                             sg["q"] * DOUT + (h + 1) * 128],
                                        ohT[sg["k"]][:, c, :],
                                        start=False, stop=False)
                            ca = p * 4
                            while ca < p * 4 + 4:
                                q0 = segs_of[ca][0]["q"]
                                cb = ca + 1
                                while cb < p * 4 + 4 and segs_of[cb][0]["q"] == q0:
                                    cb += 1
                                nc.tensor.matmul(
                                    ps[h][:, ca * 128:cb * 128],
                                    U_sb[:, q0 * DOUT + h * 128:
                                         q0 * DOUT + (h + 1) * 128],
                                    ohT[0][:, ca:cb, :],
                                    start=False, stop=True)
                                ca = cb

                    bst = sp.tile([128, 2, 24], f32, tag="bst")
                    stage_f = sp.tile([128, GB], bf16, tag="stf")
                    stage_c = sp.tile([128, GB], bf16, tag="stc")
                    for h, (stg, spl, mv) in enumerate((
                            (stage_f, spill_f, mv_f), (stage_c, spill_c, mv_c))):
                        nc.scalar.copy(stg[:], ps[h][:])
                        for p in range(4):
                            nc.vector.bn_stats(bst[:, h, p * 6:(p + 1) * 6],
                                               stg[:, p * 512:(p + 1) * 512])
                        nc.vector.bn_aggr(mv[:, 2 * bi:2 * bi + 2], bst[:, h, :])
                        nc.sync.dma_start(spl[:, b0:b0 + GB], stg[:])

            # ---------------- BN1 stats all-reduce ----------------
            # per-batch (mean, var) with equal counts (GB each, pads are zero):
            # sum = GB * sum(means); sumsq = GB * sum(var + mean^2)
            st_loc = cp.tile([128, 4], f32, tag="stloc")
            ex2b = cp.tile([128, 2 * nbatch], f32, tag="ex2b")
            for h, mv in enumerate((mv_f, mv_c)):
                means = mv.rearrange("p (n k) -> p k n", k=2)[:, 0, :]
                varls = mv.rearrange("p (n k) -> p k n", k=2)[:, 1, :]
                m2 = ex2b[:, h * nbatch:(h + 1) * nbatch]
                nc.vector.tensor_tensor(m2, means, means, ALU.mult)
                nc.vector.tensor_tensor(m2, m2, varls, ALU.add)
                nc.vector.tensor_reduce(st_loc[:, h:h + 1], means,
                                        mybir.AxisListType.X, ALU.add)
                nc.vector.tensor_reduce(st_loc[:, 2 + h:3 + h], m2,
                                        mybir.AxisListType.X, ALU.add)
            nc.vector.tensor_scalar_mul(st_loc[:], st_loc[:], float(GB))
            st_in = dram.tile([128, 4], f32)
            st_out = dram.tile([128, 4], f32)
            nc.gpsimd.dma_start(st_in[:], st_loc[:])
            nc.gpsimd.collective_compute(
                "AllReduce", ALU.add, replica_groups=[list(range(NCORES))],
                ins=[st_in.opt()], outs=[st_out.opt()],
            )
            st_g = cp.tile([128, 4], f32, tag="stg")
            nc.gpsimd.dma_start(st_g[:], st_out[:])
            # mean/var -> affine s1, t1  (b cancels in BN; never added)
            mv = cp.tile([128, 6], f32, tag="mv")
            nc.vector.tensor_scalar_mul(mv[:, 0:2], st_g[:, 0:2], 1.0 / E)
            nc.vector.tensor_scalar_mul(mv[:, 2:4], st_g[:, 2:4], 1.0 / E)
            nc.vector.tensor_tensor(mv[:, 4:6], mv[:, 0:2], mv[:, 0:2], ALU.mult)
            nc.vector.tensor_tensor(mv[:, 2:4], mv[:, 2:4], mv[:, 4:6], ALU.subtract)
            nc.vector.tensor_scalar_add(mv[:, 2:4], mv[:, 2:4], float(BN_EPS))
            std = cp.tile([128, 2], f32, tag="std")
            nc.scalar.activation(std[:], mv[:, 2:4], AF.Sqrt, bias=0.0)
            rstd = cp.tile([128, 2], f32, tag="rstd")
            nc.vector.reciprocal(rstd[:], std[:])
            nc.vector.tensor_tensor(s1[:], g1b1_sb[:, 0:2], rstd[:], ALU.mult)
            nc.vector.tensor_tensor(mv[:, 4:6], mv[:, 0:2], s1[:], ALU.mult)
            nc.vector.tensor_tensor(t1[:], g1b1_sb[:, 2:4], mv[:, 4:6], ALU.subtract)
            nc.vector.tensor_scalar_mul(ns1[:], s1[:, 0:1], -1.0)
            nc.vector.tensor_scalar_mul(nt1[:], t1[:, 0:1], -1.0)

            # ---------------- PHASE 2 ----------------
            with (
                tc.tile_pool(name="g2", bufs=3) as rp,
                tc.tile_pool(name="m2", bufs=2) as mp,
                tc.tile_pool(name="me", bufs=2) as ep,
                tc.tile_pool(name="oh2", bufs=2) as oh2,
                tc.tile_pool(name="psg", bufs=2, space="PSUM") as pg_pool,
                tc.tile_pool(name="sq2", bufs=2) as sq2p,
                tc.tile_pool(name="ps2", bufs=1, space="PSUM") as pq,
            ):
                ps_g = None
                ps_sum = pq.tile([1, 128], f32, tag="pssum")
                ps_ssq = pq.tile([1, 128], f32, tag="psssq")
                last_sub = {}
                for gb in range(nblk):
                    for sg in blocks[gb]:
                        if sg["last"]:
                            last_sub[sg["q"]] = sg["sub"]
                nq_emit = len(last_sub)
                qcount = [0]
                mb_list = list(range(0, e_pad, MB))
                for mp0 in range(0, len(mb_list), 2):
                    pair = mb_list[mp0:mp0 + 2]
                    gfs, gcs, ohs_of, sigs, msgEs = {}, {}, {}, {}, {}
                    for m0 in pair:
                        msz = min(MB, e_pad - m0)
                        gf = rp.tile([128, MB], bf16, tag="gf")
                        nc.sync.dma_start(gf[:, :msz], spill_f[:, m0:m0 + msz])
                        gc = rp.tile([128, MB], bf16, tag="gc")
                        nc.sync.dma_start(gc[:, :msz], spill_c[:, m0:m0 + msz])
                        gfs[m0], gcs[m0] = gf, gc
                        # one-hots first: no barrier dependency, DVE runs ahead
                        chunks = []
                        for c0 in range(0, msz // 128, 16):
                            gb0 = m0 // 128 + c0
                            nch = min(16, msz // 128 - c0)
                            kmax2 = max((len(blocks[gb0 + i]) for i in range(nch)),
                                        default=0)
                            ohs = []
                            for k in range(kmax2):
                                oneh = oh2.tile([128, 16, 128], bf16,
                                                name=f"oh{(c0 // 16) % 2}_{k}",
                                                tag=f"oh{(c0 // 16) % 2}_{k}")
                                nc.vector.tensor_tensor(
                                    oneh[:, :nch, :],
                                    iotar_sb[:, k * 128:(k + 1) * 128]
                                    .rearrange("p (o l) -> p o l", o=1)
                                    .to_broadcast([128, nch, 128]),
                                    slbP_sb[:, gb0:gb0 + nch]
                                    .to_broadcast([128, nch, 128]),
                                    ALU.is_equal)
                                ohs.append(oneh)
                            chunks.append((c0, nch, ohs))
                        ohs_of[m0] = chunks
                    for m0 in pair:
                        msz = min(MB, e_pad - m0)
                        sig = mp.tile([128, MB], bf16, tag="sig")
                        nc.scalar.activation(sig[:, :msz], gfs[m0][:, :msz],
                                             AF.Sigmoid,
                                             bias=t1[:, 0:1], scale=s1[:, 0:1])
                        sigs[m0] = sig
                    for m0 in pair:
                        msz = min(MB, e_pad - m0)
                        ec = mp.tile([128, MB], bf16, tag="ec")
                        nc.scalar.activation(ec[:, :msz], gcs[m0][:, :msz], AF.Exp,
                                             bias=t1[:, 1:2], scale=s1[:, 1:2])
                        # softplus = Ln(Exp(y) + 1), written over gf
                        nc.scalar.activation(gfs[m0][:, :msz], ec[:, :msz],
                                             AF.Ln, bias=1.0)
                        msgT = ec  # reuse
                        nc.vector.tensor_tensor(msgT[:, :msz], sigs[m0][:, :msz],
                                                gfs[m0][:, :msz], ALU.mult)
                        msgE = ep.tile([128, MB // 128, 128], bf16, tag="msgE")
                        nc.sync.dma_start_transpose(msgE[:, :msz // 128, :],
                                                    msgT[:, :msz])
                        msgEs[m0] = msgE
                    for m0 in pair:
                        msz = min(MB, e_pad - m0)
                        for c0, nch, ohs in ohs_of[m0]:
                            for ci in range(nch):
                                gb = m0 // 128 + c0 + ci
                                for sg in blocks[gb]:
                                    if sg["first"]:
                                        ps_g = pg_pool.tile([128, 128], f32,
                                                            tag="psg")
                                    nc.tensor.matmul(
                                        ps_g[:], ohs[sg["k"]][:, ci, :],
                                        msgEs[m0][:, c0 + ci, :],
                                        start=sg["first"], stop=sg["last"])
                                    if sg["last"]:
                                        q = sg["q"]
                                        nc.vector.tensor_tensor(
                                            summed[:, q * 128:(q + 1) * 128],
                                            summed[:, q * 128:(q + 1) * 128],
                                            ps_g[:], ALU.add)
                                        if last_sub[q] == sg["sub"]:
                                            sq = sq2p.tile([128, 128], f32,
                                                           tag="sq")
                                            nc.vector.tensor_tensor(
                                                sq[:],
                                                summed[:, q * 128:(q + 1) * 128],
                                                summed[:, q * 128:(q + 1) * 128],
                                                ALU.mult)
                                            i = qcount[0]; qcount[0] += 1
                                            nc.tensor.matmul(
                                                ps_sum[:], ones_c[:],
                                                summed[:, q * 128:(q + 1) * 128],
                                                start=(i == 0),
                                                stop=(i == nq_emit - 1))
                                            nc.tensor.matmul(
                                                ps_ssq[:], ones_c[:], sq[:],
                                                start=(i == 0),
                                                stop=(i == nq_emit - 1))

                # ---------------- BN2 finalize ----------------
                st2 = cp.tile([1, 256], f32, tag="st2")
                nc.scalar.copy(st2[:, 0:128], ps_sum[:])
                nc.scalar.copy(st2[:, 128:256], ps_ssq[:])
                st2_in = dram.tile([1, 256], f32)
                st2_out = dram.tile([1, 256], f32)
                nc.gpsimd.dma_start(st2_in[:], st2[:])
                nc.gpsimd.collective_compute(
                    "AllReduce", ALU.add, replica_groups=[list(range(NCORES))],
                    ins=[st2_in.opt()], outs=[st2_out.opt()],
                )
                st2g = cp.tile([1, 256], f32, tag="st2g")
                nc.gpsimd.dma_start(st2g[:], st2_out[:])
                mv2 = cp.tile([1, 384], f32, tag="mv2")
                nc.vector.tensor_scalar_mul(mv2[:, 0:256], st2g[:], 1.0 / N)
                nc.vector.tensor_tensor(mv2[:, 256:384], mv2[:, 0:128],
                                        mv2[:, 0:128], ALU.mult)
                nc.vector.tensor_tensor(mv2[:, 128:256], mv2[:, 128:256],
                                        mv2[:, 256:384], ALU.subtract)
                nc.vector.tensor_scalar_add(mv2[:, 128:256], mv2[:, 128:256],
                                            float(BN_EPS))
                std2 = cp.tile([1, 128], f32, tag="std2")
                nc.scalar.activation(std2[:], mv2[:, 128:256], AF.Sqrt, bias=0.0)
                rstd2 = cp.tile([1, 128], f32, tag="rstd2")
                nc.vector.reciprocal(rstd2[:], std2[:])
                strow = cp.tile([1, 256], f32, tag="strow")
                nc.vector.tensor_tensor(strow[:, 0:128], g2b2_sb[:, 0:128],
                                        rstd2[:], ALU.mult)
                nc.vector.tensor_tensor(mv2[:, 256:384], mv2[:, 0:128],
                                        strow[:, 0:128], ALU.mult)
                nc.vector.tensor_tensor(strow[:, 128:256], g2b2_sb[:, 128:256],
                                        mv2[:, 256:384], ALU.subtract)
                ps_bc = pq.tile([128, 256], f32, tag="psbc")
                nc.tensor.matmul(ps_bc[:], ones_r[:], strow[:], start=True, stop=True)
                s2t2 = cp.tile([128, 256], f32, tag="s2t2")
                nc.scalar.copy(s2t2[:], ps_bc[:])
                y3 = y_d.rearrange("(q p) f -> p q f", p=128)
                sm3 = summed.rearrange("p (q l) -> p q l", l=128)
                for q0 in range(0, GPC, 7):
                    og = sq2p.tile([128, 7, 128], f32, name="og", tag="og")
                    nc.vector.tensor_tensor(
                        og[:], sm3[:, q0:q0 + 7, :],
                        s2t2[:, 0:128].rearrange("p (o l) -> p o l", o=1)
                        .to_broadcast([128, 7, 128]), ALU.mult)
                    nc.vector.tensor_tensor(
                        og[:], og[:],
                        s2t2[:, 128:256].rearrange("p (o l) -> p o l", o=1)
                        .to_broadcast([128, 7, 128]), ALU.add)
                    nc.sync.dma_start(y3[:, q0:q0 + 7, :], og[:])
    nc.compile()
    return nc


def _make_in_maps(per_core, struct, inputs):
    max_k = max(2, struct["max_k"])
    g1 = np.asarray(inputs["gamma1"], np.float32).reshape(2, 128).T  # [128,2]
    b1 = np.asarray(inputs["beta1"], np.float32).reshape(2, 128).T
    g1b1 = np.ascontiguousarray(np.concatenate([g1, b1], axis=1))  # [128,4]
    g2b2 = np.concatenate([np.asarray(inputs["gamma2"], np.float32),
                           np.asarray(inputs["beta2"], np.float32)]).reshape(1, 256)
    iotac = (np.arange(128, dtype=np.float32)[:, None]
             + 128.0 * np.arange(max_k, dtype=np.float32)[None, :])
    iotar = np.tile(np.arange(max_k * 128, dtype=np.float32), (128, 1)).astype(FP16)
    shared = dict(
        wt=np.asarray(inputs["W"], np.float32).astype(BF16),
        g1b1=g1b1,
        g2b2=np.ascontiguousarray(g2b2),
        iden=np.eye(128, dtype=FP16),
        iotac=np.ascontiguousarray(iotac),
        iotar=np.ascontiguousarray(iotar),
        ones_col=np.ones((128, 1), np.float32),
        ones_row=np.ones((1, 128), np.float32),
    )
    return [{**pc, **shared} for pc in per_core]


def kernel(x, edge_index, edge_attr, W, b, gamma1, beta1, gamma2, beta2):
    per_core, struct = _prep(x, edge_index, edge_attr)
    in_maps = _make_in_maps(
        per_core, struct,
        dict(W=W, gamma1=gamma1, beta1=beta1, gamma2=gamma2, beta2=beta2),
    )
    nc = _build(struct)
    res = bass_utils.run_bass_kernel_spmd(nc, in_maps, core_ids=list(range(NCORES)))
    out = np.concatenate([res.results[c]["y"] for c in range(NCORES)], axis=0)
    return np.ascontiguousarray(out[:N])


if __name__ == "__main__":
    import reference

    inputs = {k: np.asarray(v) for k, v in reference.setup_inputs().items()}
    got = kernel(**inputs)
    exp = np.asarray(reference.reference(**inputs))
    err = np.abs(got - exp).max() / np.abs(exp).max()
    print("rel err:", err)
